# revision 1
# baseline (speedup 1.0000x reference)
import sys
sys.path.insert(0, '/opt/trn_rl_repo')
import numpy as np
import concourse.bass as bass
import concourse.bacc as bacc
import concourse.tile as tile
from concourse import mybir
from concourse.bass_utils import run_bass_kernel_spmd

F32 = mybir.dt.float32
AX = mybir.AxisListType
AF = mybir.ActivationFunctionType

T, B, CK, CV, H, W = 4, 2, 256, 512, 96, 96
HW = H * W                 # 9216
NCORES = 8
CKC = CK // NCORES         # 32 key channels per core
CVC = CV // NCORES         # 64 value channels per core
NS = HW // NCORES          # 1152 query positions per core
NCH = 3                    # n-chunks per shard
NW = NS // NCH             # 384 (fits one PSUM bank)
# (si, s, row offset of scale block inside the 50 pooled values)
SCALES = [(0, 1, 49), (1, 2, 45), (2, 3, 36), (3, 6, 0)]
BLK = {1: 9216, 2: 2304, 3: 1024, 6: 256}  # elements averaged per pooled value


def _resize_mat(s):
    # 1D bilinear (half-pixel centers, triangle kernel, row-normalized) —
    # matches jax.image.resize(method='bilinear') for upsampling s -> 6.
    out = np.zeros((6, s), np.float64)
    for i in range(6):
        x = (i + 0.5) * s / 6.0 - 0.5
        for p in range(s):
            out[i, p] = max(0.0, 1.0 - abs(x - p))
        out[i] /= out[i].sum()
    return out


def _upsample_mats():
    mats = []
    for si, s, off in SCALES:
        R = _resize_mat(s)  # [6, s]
        U = np.einsum('ip,jq->pqij', R, R).reshape(s * s, 36)
        mats.append(np.ascontiguousarray(U.astype(np.float32)))
    return mats


_BUILT = {}


def _build_nc(repeats=1, phase='full'):
    nc = bacc.Bacc("TRN2", target_bir_lowering=False, debug=False,
                   num_devices=NCORES)
    mk = nc.dram_tensor("mk", [T, B, CKC, HW], F32, kind="ExternalInput")
    mv = nc.dram_tensor("mv", [T, B, CVC, HW], F32, kind="ExternalInput")
    qk = nc.dram_tensor("qk", [B, CK, NS], F32, kind="ExternalInput")
    qv = nc.dram_tensor("qv", [B, CV, NS], F32, kind="ExternalInput")
    # weight chunks replicated vertically so rhs base_partition can match
    # the pooled lhsT slice (matmul requires equal base partitions)
    wk = nc.dram_tensor("wk", [T, 128, CK], F32, kind="ExternalInput")
    wv = nc.dram_tensor("wv", [T, 128, CV], F32, kind="ExternalInput")
    bkd = nc.dram_tensor("bk", [50, T * B * 64], F32, kind="ExternalInput")
    bvd = nc.dram_tensor("bv", [50, T * B * 128], F32, kind="ExternalInput")
    uds = [nc.dram_tensor(f"u{si}", [s * s, 36], F32, kind="ExternalInput")
           for si, s, off in SCALES]
    out = nc.dram_tensor("out", [B, 2 * CV, NS], F32, kind="ExternalOutput")

    with tile.TileContext(nc) as tc:
        with (
            tc.tile_pool(name="inp", bufs=2) as inp,
            tc.tile_pool(name="s1p", bufs=2) as s1p,
            tc.tile_pool(name="plp", bufs=6) as plp,
            tc.tile_pool(name="cst", bufs=1) as cst,
            tc.tile_pool(name="rlp", bufs=1) as rlp,
            tc.tile_pool(name="bnk", bufs=1) as bnk,
            tc.tile_pool(name="qkp", bufs=1) as qkp,
            tc.tile_pool(name="exp", bufs=8) as expp,
            tc.tile_pool(name="smm", bufs=2) as smm,
            tc.tile_pool(name="bsp", bufs=2) as bsp,
            tc.tile_pool(name="cvs", bufs=3) as cvs,
            tc.tile_pool(name="omp", bufs=3) as omp,
            tc.tile_pool(name="ps", bufs=4, space="PSUM") as psp,
            tc.tile_pool(name="pss", bufs=2, space="PSUM") as pss,
            tc.tile_pool(name="pbc", bufs=2, space="PSUM") as pbc,
            tc.tile_pool(name="dram", bufs=1, space="DRAM") as drp,
        ):
            ones = cst.tile([128, 128], F32, tag="ones")
            nc.vector.memset(ones[:], 1.0)
            u_sb = []
            for (si, s, off), ud in zip(SCALES, uds):
                ut = cst.tile([s * s, 36], F32, tag=f"u{si}")
                nc.sync.dma_start(ut[:], ud[:])
                u_sb.append(ut)
            wk_sb, wv_sb = [], []
            for t in range(T):
                wt = cst.tile([128, CK], F32, tag=f"wk{t}")
                nc.sync.dma_start(wt[:], wk[t])
                wk_sb.append(wt)
                vt = cst.tile([128, CV], F32, tag=f"wv{t}")
                nc.sync.dma_start(vt[:], wv[t])
                wv_sb.append(vt)

            cbuf = drp.tile([T, B, 50, 192], F32, tag="cbuf")
            arout = drp.tile([T, B, 50, 192], F32, tag="arout")

            for _rep in range(repeats):
                # ---- Phase A/B: pooling + partial conv, per 128-map tile ----
                map_tiles = []
                for i in range(2):  # keys: (t,b,c) rows 128*i .. 128*i+128
                    src = mk[2 * i:2 * i + 2].rearrange("t b c n -> (t b c) n")
                    groups = [(2 * i + g // 2, g % 2, 32 * g, 32, 'k')
                              for g in range(4)]
                    map_tiles.append((src, groups))
                for j in range(4):  # values
                    src = mv[j:j + 1].rearrange("t b c n -> (t b c) n")
                    groups = [(j, g, 64 * g, 64, 'v') for g in range(2)]
                    map_tiles.append((src, groups))

                for src, groups in map_tiles:
                    xt = inp.tile([128, HW], F32, tag="xt")
                    nc.sync.dma_start(xt[:], src[:])
                    s1 = s1p.tile([128, 576], F32, tag="s1")
                    nc.vector.reduce_sum(
                        s1[:],
                        xt[:].rearrange("p (h wb wi) -> p h wb wi",
                                        h=96, wb=6, wi=16),
                        axis=AX.X)
                    pl = plp.tile([128, 50], F32, tag="pl")
                    nc.vector.reduce_sum(
                        pl[:, 0:36],
                        s1[:].rearrange("p (hb hi wb) -> p hb wb hi",
                                        hb=6, hi=16, wb=6),
                        axis=AX.X)
                    nc.vector.reduce_sum(
                        pl[:, 36:45],
                        pl[:, 0:36].rearrange("p (a i b j) -> p a b i j",
                                              a=3, i=2, b=3, j=2),
                        axis=AX.XY)
                    nc.vector.reduce_sum(
                        pl[:, 45:49],
                        pl[:, 0:36].rearrange("p (a i b j) -> p a b i j",
                                              a=2, i=3, b=2, j=3),
                        axis=AX.XY)
                    nc.vector.reduce_sum(pl[:, 49:50], pl[:, 0:36], axis=AX.X)
                    for (t, b, p0, kp, kind) in groups:
                        tp = (p0, 0) if p0 == 96 else None
                        if kind == 'k':
                            cps = psp.tile([50, CK], F32, tag="ps")
                            nc.tensor.matmul(cps[:], pl[p0:p0 + kp, :],
                                             wk_sb[t][p0:p0 + kp, :],
                                             start=True, stop=True,
                                             tile_position=tp)
                            stg = cvs.tile([50, CK], F32, tag="cvs")
                            nc.scalar.copy(stg[:], cps[:])
                            for si, s, off in SCALES:
                                s2 = s * s
                                nc.sync.dma_start(
                                    cbuf[t, b, off:off + s2, 0:64],
                                    stg[off:off + s2, si * 64:(si + 1) * 64])
                        else:
                            cps = psp.tile([50, CV], F32, tag="ps")
                            nc.tensor.matmul(cps[:], pl[p0:p0 + kp, :],
                                             wv_sb[t][p0:p0 + kp, :],
                                             start=True, stop=True,
                                             tile_position=tp)
                            stg = cvs.tile([50, CV], F32, tag="cvs")
                            nc.scalar.copy(stg[:], cps[:])
                            for si, s, off in SCALES:
                                s2 = s * s
                                nc.sync.dma_start(
                                    cbuf[t, b, off:off + s2, 64:192],
                                    stg[off:off + s2, si * 128:(si + 1) * 128])

                if phase == 'pool':
                    for b in range(B):
                        nc.sync.dma_start(out[b, 0:512, :], qv[b])
                    continue
                # ---- AllReduce of partial conv outputs (307 KB) ----
                if phase == 'noar':
                    nc.sync.dma_start(arout[:], cbuf[:])
                else:
                    nc.gpsimd.collective_compute(
                        "AllReduce", mybir.AluOpType.add,
                        replica_groups=[list(range(NCORES))],
                        ins=[cbuf.opt()], outs=[arout.opt()])

                # ---- Phase C: bias + relu (per scale, base partition 0) ----
                rk_sb, rv_sb = {}, {}
                for si, s, off in SCALES:
                    s2 = s * s
                    rk_t = rlp.tile([s2, T * B * 64], F32, tag=f"rk{si}")
                    nc.sync.dma_start(
                        rk_t[:],
                        arout[:, :, off:off + s2, 0:64].rearrange(
                            "t b v o -> v t b o"))
                    bkt = bsp.tile([s2, T * B * 64], F32, name="bkt", tag="bk")
                    nc.sync.dma_start(bkt[:], bkd[off:off + s2, :])
                    nc.vector.tensor_add(rk_t[:], rk_t[:], bkt[:])
                    nc.vector.tensor_scalar_max(rk_t[:], rk_t[:], 0.0)
                    rk_sb[si] = rk_t
                    rv_t = rlp.tile([s2, T * B * 128], F32, tag=f"rv{si}")
                    nc.sync.dma_start(
                        rv_t[:],
                        arout[:, :, off:off + s2, 64:192].rearrange(
                            "t b v o -> v t b o"))
                    bvt = bsp.tile([s2, T * B * 128], F32, name="bvt", tag="bv")
                    nc.sync.dma_start(bvt[:], bvd[off:off + s2, :])
                    nc.vector.tensor_add(rv_t[:], rv_t[:], bvt[:])
                    nc.vector.tensor_scalar_max(rv_t[:], rv_t[:], 0.0)
                    rv_sb[si] = rv_t

                # ---- Phase D: build memory banks via upsample matmuls ----
                # bank_k halves: [c-half(128), m(144)] — lhsT for att (K=128)
                bkh = [[bnk.tile([128, 144], F32, name=f"bkh{b}{h}",
                                 tag=f"bkh{b}{h}") for h in range(2)]
                       for b in range(B)]
                for b in range(B):
                    for si, s, off in SCALES:
                        ps_bk = psp.tile([64, 144], F32, tag="ps")
                        for t in range(T):
                            g = t * 2 + b
                            nc.tensor.matmul(
                                ps_bk[:, t * 36:(t + 1) * 36],
                                rk_sb[si][:, g * 64:(g + 1) * 64],
                                u_sb[si][:], start=True, stop=True)
                        stg = smm.tile([64, 144], F32, name="stgk", tag="stgk")
                        nc.vector.tensor_copy(stg[:], ps_bk[:])
                        nc.sync.dma_start(
                            bkh[b][si // 2][(si % 2) * 64:(si % 2) * 64 + 64, :],
                            stg[:])
                # bank_v m-tiles: bv0 [m 0:128, v 512], bv1 [m 128:144, v 512]
                bv0 = [bnk.tile([128, 512], F32, name=f"bv0{b}", tag=f"bv0{b}")
                       for b in range(B)]
                bv1 = [bnk.tile([16, 512], F32, name=f"bv1{b}", tag=f"bv1{b}")
                       for b in range(B)]
                for b in range(B):
                    for t in range(T):
                        g = t * 2 + b
                        ps_bv = psp.tile([36, 512], F32, tag="ps")
                        for si, s, off in SCALES:
                            nc.tensor.matmul(
                                ps_bv[:, si * 128:(si + 1) * 128],
                                u_sb[si][:],
                                rv_sb[si][:, g * 128:(g + 1) * 128],
                                start=True, stop=True)
                        stg = smm.tile([36, 512], F32, name="stgv", tag="stgv")
                        nc.vector.tensor_copy(stg[:], ps_bv[:])
                        if t < 3:
                            nc.sync.dma_start(
                                bv0[b][t * 36:(t + 1) * 36, :], stg[:])
                        else:
                            nc.sync.dma_start(bv0[b][108:128, :], stg[0:20, :])
                            nc.sync.dma_start(bv1[b][0:16, :], stg[20:36, :])

                # ---- query loads + query_value passthrough ----
                qkh = [[qkp.tile([128, NS], F32, name=f"qkh{b}{h}",
                                 tag=f"qkh{b}{h}") for h in range(2)]
                       for b in range(B)]
                for b in range(B):
                    for h in range(2):
                        nc.sync.dma_start(qkh[b][h][:],
                                          qk[b, 128 * h:128 * (h + 1), :])
                    nc.sync.dma_start(out[b, 0:512, :], qv[b])

                # ---- Phase E: attention (m split 128+16) ----
                for b in range(B):
                    for ch in range(NCH):
                        cs = slice(ch * NW, (ch + 1) * NW)
                        aps0 = psp.tile([128, NW], F32, tag="ps")
                        aps1 = psp.tile([16, NW], F32, tag="ps")
                        for h in range(2):
                            nc.tensor.matmul(aps0[:], bkh[b][h][:, 0:128],
                                             qkh[b][h][:, cs],
                                             start=(h == 0), stop=(h == 1))
                        for h in range(2):
                            nc.tensor.matmul(aps1[:], bkh[b][h][:, 128:144],
                                             qkh[b][h][:, cs],
                                             start=(h == 0), stop=(h == 1))
                        ex0 = expp.tile([128, NW], F32, name="ex0", tag="ex0")
                        ex1 = expp.tile([16, NW], F32, name="ex1", tag="ex1")
                        nc.scalar.activation(ex0[:], aps0[:], AF.Exp,
                                             scale=1.0 / 16.0)
                        nc.scalar.activation(ex1[:], aps1[:], AF.Exp,
                                             scale=1.0 / 16.0)
                        sums = pss.tile([1, NW], F32, tag="sums")
                        nc.tensor.matmul(sums[:], ones[0:128, 0:1], ex0[:],
                                         start=True, stop=False)
                        nc.tensor.matmul(sums[:], ones[0:16, 0:1], ex1[:],
                                         start=False, stop=True)
                        srow = smm.tile([1, NW], F32, name="srow", tag="srow")
                        nc.vector.tensor_copy(srow[:], sums[:])
                        spk = smm.tile([16, 24], F32, name="spk", tag="spk")
                        nc.sync.dma_start(spk[:], srow[:])
                        rcp = smm.tile([16, 24], F32, name="rcp", tag="rcp")
                        nc.vector.reciprocal(rcp[:], spk[:])
                        rrow = smm.tile([1, NW], F32, name="rrow", tag="rr")
                        nc.sync.dma_start(rrow[:], rcp[:])
                        bc = pbc.tile([128, NW], F32, tag="bc")
                        nc.tensor.matmul(bc[:], ones[0:1, 0:128], rrow[:],
                                         start=True, stop=True)
                        bc_sb = smm.tile([128, NW], F32, name="bc_sb",
                                         tag="bcs")
                        nc.scalar.copy(bc_sb[:], bc[:])
                        for vc in range(4):
                            vs = slice(vc * 128, (vc + 1) * 128)
                            mps = psp.tile([128, NW], F32, tag="ps")
                            nc.tensor.matmul(mps[:], bv0[b][:, vs], ex0[:],
                                             start=True, stop=False)
                            nc.tensor.matmul(mps[:], bv1[b][:, vs], ex1[:],
                                             start=False, stop=True)
                            om = omp.tile([128, NW], F32, tag="om")
                            nc.vector.tensor_mul(om[:], mps[:], bc_sb[:])
                            nc.sync.dma_start(
                                out[b, 512 + vc * 128:512 + (vc + 1) * 128,
                                    cs],
                                om[:])

    nc.compile()
    return nc


def _get_nc(repeats=1, phase='full'):
    key = (repeats, phase)
    if key not in _BUILT:
        _BUILT[key] = _build_nc(repeats, phase)
    return _BUILT[key]


def _host_prep(memory_keys, memory_values, query_key, query_value,
               key_w, key_b, val_w, val_b):
    mk = np.asarray(memory_keys, np.float32).reshape(T, B, CK, HW)
    mv = np.asarray(memory_values, np.float32).reshape(T, B, CV, HW)
    qk = np.asarray(query_key, np.float32).reshape(B, CK, HW)
    qv = np.asarray(query_value, np.float32).reshape(B, CV, HW)
    kw = np.asarray(key_w, np.float32).copy()
    vw = np.asarray(val_w, np.float32).copy()
    kb = np.asarray(key_b, np.float32)
    vb = np.asarray(val_b, np.float32)
    for si, s, off in SCALES:
        kw[:, si] /= BLK[s]
        vw[:, si] /= BLK[s]

    bk_host = np.zeros((50, T * B * 64), np.float32)
    bv_host = np.zeros((50, T * B * 128), np.float32)
    for si, s, off in SCALES:
        s2 = s * s
        for t in range(T):
            for b in range(B):
                g = t * 2 + b
                bk_host[off:off + s2, g * 64:(g + 1) * 64] = kb[t, si][None, :]
                bv_host[off:off + s2, g * 128:(g + 1) * 128] = vb[t, si][None, :]
    umats = _upsample_mats()

    in_maps = []
    for k in range(NCORES):
        m = {
            "mk": np.ascontiguousarray(mk[:, :, k * CKC:(k + 1) * CKC, :]),
            "mv": np.ascontiguousarray(mv[:, :, k * CVC:(k + 1) * CVC, :]),
            "qk": np.ascontiguousarray(qk[:, :, k * NS:(k + 1) * NS]),
            "qv": np.ascontiguousarray(qv[:, :, k * NS:(k + 1) * NS]),
            # wk[t, c, si*64+o] = key_w[t, si, o, chunk_c] / blk
            "wk": np.ascontiguousarray(np.tile(
                kw[:, :, :, k * CKC:(k + 1) * CKC]
                .transpose(0, 3, 1, 2).reshape(T, CKC, CK), (1, 4, 1))),
            "wv": np.ascontiguousarray(np.tile(
                vw[:, :, :, k * CVC:(k + 1) * CVC]
                .transpose(0, 3, 1, 2).reshape(T, CVC, CV), (1, 2, 1))),
            "bk": bk_host, "bv": bv_host,
        }
        for (si, s, off), u in zip(SCALES, umats):
            m[f"u{si}"] = u
        in_maps.append(m)
    return in_maps


def kernel(**inputs):
    nc = _get_nc()
    in_maps = _host_prep(**inputs)
    res = run_bass_kernel_spmd(nc, in_maps, core_ids=list(range(NCORES)),
                               trace=False)
    shards = [res.results[i]["out"] for i in range(NCORES)]
    full = np.concatenate(shards, axis=2).reshape(B, 2 * CV, H, W)
    return full



# revision 12
# speedup vs baseline: 3.2078x; 3.2078x over previous
import sys
sys.path.insert(0, '/opt/trn_rl_repo')
import numpy as np
import concourse.bass as bass
import concourse.bacc as bacc
import concourse.tile as tile
from concourse import mybir
from concourse.bass_utils import run_bass_kernel_spmd

F32 = mybir.dt.float32
BF16 = mybir.dt.bfloat16
NPBF = mybir.dt.np(BF16)
AX = mybir.AxisListType
AF = mybir.ActivationFunctionType

T, B, CK, CV, H, W = 4, 2, 256, 512, 96, 96
HW = H * W                 # 9216
NCORES = 8
CKC = CK // NCORES         # 32 key channels per core
CVC = CV // NCORES         # 64 value channels per core
NS = HW // NCORES          # 1152 query positions per core
NCH = 3                    # n-chunks per shard
NW = NS // NCH             # 384 (fits one PSUM bank)
# (si, s, row offset of scale block inside the 50 pooled values)
SCALES = [(0, 1, 49), (1, 2, 45), (2, 3, 36), (3, 6, 0)]
BLK = {1: 9216, 2: 2304, 3: 1024, 6: 256}  # elements averaged per pooled value


def _resize_mat(s):
    # 1D bilinear (half-pixel centers, triangle kernel, row-normalized) —
    # matches jax.image.resize(method='bilinear') for upsampling s -> 6.
    out = np.zeros((6, s), np.float64)
    for i in range(6):
        x = (i + 0.5) * s / 6.0 - 0.5
        for p in range(s):
            out[i, p] = max(0.0, 1.0 - abs(x - p))
        out[i] /= out[i].sum()
    return out


def _upsample_mats():
    mats = []
    for si, s, off in SCALES:
        R = _resize_mat(s)  # [6, s]
        U = np.einsum('ip,jq->pqij', R, R).reshape(s * s, 36)
        mats.append(np.ascontiguousarray(U.astype(np.float32)))
    return mats


_BUILT = {}


def _build_nc(repeats=1, phase='full'):
    nc = bacc.Bacc("TRN2", target_bir_lowering=False, debug=False,
                   num_devices=NCORES)
    mk = nc.dram_tensor("mk", [T, B, CKC, HW], BF16, kind="ExternalInput")
    mv = nc.dram_tensor("mv", [T, B, CVC, HW], BF16, kind="ExternalInput")
    qk = nc.dram_tensor("qk", [B, CK, NS], BF16, kind="ExternalInput")
    qv = nc.dram_tensor("qv", [B, CV, NS], BF16, kind="ExternalInput")
    # weight chunks replicated vertically so rhs base_partition can match
    # the pooled lhsT slice (matmul requires equal base partitions)
    wk = nc.dram_tensor("wk", [T, 128, CK], BF16, kind="ExternalInput")
    wv = nc.dram_tensor("wv", [T, 128, CV], BF16, kind="ExternalInput")
    # combined bias constant per scale block: [s2, 512 (k) + 1024 (v)]
    bcd = nc.dram_tensor("bc", [50, 1536], BF16, kind="ExternalInput")
    uds = [nc.dram_tensor(f"u{si}", [s * s, 36], BF16, kind="ExternalInput")
           for si, s, off in SCALES]
    out = nc.dram_tensor("out", [B, 2 * CV, NS], BF16, kind="ExternalOutput")

    from contextlib import ExitStack
    with tile.TileContext(nc) as tc:
        with ExitStack() as _stk:
            ent = _stk.enter_context
            ent(nc.allow_low_precision(
                reason="bf16 pipeline; problem tolerance is 2e-2 rel"))
            inp = ent(tc.tile_pool(name="inp", bufs=2))
            tre = ent(tc.tile_pool(name="tre", bufs=1))
            plp = ent(tc.tile_pool(name="plp", bufs=6))
            cst = ent(tc.tile_pool(name="cst", bufs=1))
            stgp = ent(tc.tile_pool(name="stg", bufs=2))
            rlp = ent(tc.tile_pool(name="rlp", bufs=2))
            bnk = ent(tc.tile_pool(name="bnk", bufs=2))
            qkp = ent(tc.tile_pool(name="qkp", bufs=2))
            expp = ent(tc.tile_pool(name="exp", bufs=4))
            smm = ent(tc.tile_pool(name="smm", bufs=3))
            cvs = ent(tc.tile_pool(name="cvs", bufs=3))
            omp = ent(tc.tile_pool(name="omp", bufs=2))
            psp = ent(tc.tile_pool(name="ps", bufs=4, space="PSUM"))
            pss = ent(tc.tile_pool(name="pss", bufs=2, space="PSUM"))
            pbc = ent(tc.tile_pool(name="pbc", bufs=2, space="PSUM"))
            drp = ent(tc.tile_pool(name="dram", bufs=2, space="DRAM"))

            ones = cst.tile([128, 128], BF16, tag="ones")
            nc.vector.memset(ones[:], 1.0)
            u_sb = []
            for (si, s, off), ud in zip(SCALES, uds):
                ut = cst.tile([s * s, 36], BF16, tag=f"u{si}")
                nc.sync.dma_start(ut[:], ud[:])
                u_sb.append(ut)
            wk_sb, wv_sb = [], []
            for t in range(T):
                wt = cst.tile([128, CK], BF16, tag=f"wk{t}")
                nc.sync.dma_start(wt[:], wk[t])
                wk_sb.append(wt)
                vt = cst.tile([128, CV], BF16, tag=f"wv{t}")
                nc.sync.dma_start(vt[:], wv[t])
                wv_sb.append(vt)
            # per-scale combined bias tiles [s2, 1536]
            bias_sb = {}
            for si, s, off in SCALES:
                s2 = s * s
                bt = cst.tile([s2, 1536], BF16, tag=f"bias{si}")
                nc.sync.dma_start(bt[:], bcd[off:off + s2, :])
                bias_sb[si] = bt

            for _rep in range(repeats):
                cbuf = drp.tile([50, 1536], BF16, tag="cbuf")
                arout = drp.tile([50, 1536], BF16, tag="arout")
                # per-scale staging for partial conv outputs, all (t,b)
                # groups side by side: keys cols g*64, values g*128
                stgk = {}
                stgv = {}
                for si, s, off in SCALES:
                    s2 = s * s
                    stgk[si] = stgp.tile([s2, 512], BF16,
                                         name=f"stgk{si}", tag=f"stgk{si}")
                    stgv[si] = stgp.tile([s2, 1024], BF16,
                                         name=f"stgv{si}", tag=f"stgv{si}")

                # ---- Phase A: pooling (DVE pairwise tree) + partial conv ----
                map_tiles = []
                for i in range(2):  # keys: (t,b,c) rows 128*i .. 128*i+128
                    src = mk[2 * i:2 * i + 2].rearrange("t b c n -> (t b c) n")
                    groups = [(2 * i + g // 2, g % 2, 32 * g, 32, 'k')
                              for g in range(4)]
                    map_tiles.append((src, groups))
                for j in range(4):  # values
                    src = mv[j:j + 1].rearrange("t b c n -> (t b c) n")
                    groups = [(j, g, 64 * g, 64, 'v') for g in range(2)]
                    map_tiles.append((src, groups))

                for src, groups in map_tiles:
                    xt = inp.tile([128, HW], BF16, tag="xt")
                    nc.sync.dma_start(xt[:], src[:])
                    # W-direction tree: 16 -> 8 -> 4 -> 2 -> 1
                    t1 = tre.tile([128, 4608], BF16, tag="t1")
                    xv = xt[:].rearrange("p (a i) -> p a i", a=576, i=16)
                    nc.vector.tensor_add(
                        t1[:].rearrange("p (a i) -> p a i", a=576, i=8),
                        xv[:, :, 0:8], xv[:, :, 8:16])
                    t2 = tre.tile([128, 2304], BF16, tag="t2")
                    t1v = t1[:].rearrange("p (a i) -> p a i", a=576, i=8)
                    nc.vector.tensor_add(
                        t2[:].rearrange("p (a i) -> p a i", a=576, i=4),
                        t1v[:, :, 0:4], t1v[:, :, 4:8])
                    t3 = tre.tile([128, 1152], BF16, tag="t3")
                    t2v = t2[:].rearrange("p (a i) -> p a i", a=576, i=4)
                    nc.vector.tensor_add(
                        t3[:].rearrange("p (a i) -> p a i", a=576, i=2),
                        t2v[:, :, 0:2], t2v[:, :, 2:4])
                    s1 = tre.tile([128, 576], BF16, tag="s1")
                    t3v = t3[:].rearrange("p (a i) -> p a i", a=576, i=2)
                    nc.vector.tensor_add(
                        s1[:].rearrange("p (a i) -> p a i", a=576, i=1),
                        t3v[:, :, 0:1], t3v[:, :, 1:2])
                    # H-direction tree on [p, hb6, hi16, wb6]: 16->8->4->2->1
                    h1 = tre.tile([128, 288], BF16, tag="h1")
                    s1v = s1[:].rearrange("p (a i b) -> p a i b", a=6, i=16, b=6)
                    nc.vector.tensor_add(
                        h1[:].rearrange("p (a i b) -> p a i b", a=6, i=8, b=6),
                        s1v[:, :, 0:8, :], s1v[:, :, 8:16, :])
                    h2 = tre.tile([128, 144], BF16, tag="h2")
                    h1v = h1[:].rearrange("p (a i b) -> p a i b", a=6, i=8, b=6)
                    nc.vector.tensor_add(
                        h2[:].rearrange("p (a i b) -> p a i b", a=6, i=4, b=6),
                        h1v[:, :, 0:4, :], h1v[:, :, 4:8, :])
                    h3 = tre.tile([128, 72], BF16, tag="h3")
                    h2v = h2[:].rearrange("p (a i b) -> p a i b", a=6, i=4, b=6)
                    nc.vector.tensor_add(
                        h3[:].rearrange("p (a i b) -> p a i b", a=6, i=2, b=6),
                        h2v[:, :, 0:2, :], h2v[:, :, 2:4, :])
                    pl = plp.tile([128, 50], BF16, tag="pl")
                    h3v = h3[:].rearrange("p (a i b) -> p a i b", a=6, i=2, b=6)
                    nc.vector.tensor_add(
                        pl[:, 0:36].rearrange("p (a i b) -> p a i b",
                                              a=6, i=1, b=6),
                        h3v[:, :, 0:1, :], h3v[:, :, 1:2, :])
                    # coarser scales from the 6x6 block sums
                    nc.vector.reduce_sum(
                        pl[:, 36:45],
                        pl[:, 0:36].rearrange("p (a i b j) -> p a b i j",
                                              a=3, i=2, b=3, j=2),
                        axis=AX.XY)
                    nc.vector.reduce_sum(
                        pl[:, 45:49],
                        pl[:, 0:36].rearrange("p (a i b j) -> p a b i j",
                                              a=2, i=3, b=2, j=3),
                        axis=AX.XY)
                    nc.vector.reduce_sum(pl[:, 49:50], pl[:, 0:36], axis=AX.X)
                    for (t, b, p0, kp, kind) in groups:
                        tp = (p0, 0) if p0 == 96 else None
                        g = t * 2 + b
                        if kind == 'k':
                            for si, s, off in SCALES:
                                s2 = s * s
                                cps = psp.tile([s2, 64], F32, tag="ps")
                                nc.tensor.matmul(
                                    cps[:], pl[p0:p0 + kp, off:off + s2],
                                    wk_sb[t][p0:p0 + kp,
                                             si * 64:(si + 1) * 64],
                                    start=True, stop=True, tile_position=tp)
                                nc.scalar.copy(
                                    stgk[si][:, g * 64:(g + 1) * 64], cps[:])
                        else:
                            for si, s, off in SCALES:
                                s2 = s * s
                                cps = psp.tile([s2, 128], F32, tag="ps")
                                nc.tensor.matmul(
                                    cps[:], pl[p0:p0 + kp, off:off + s2],
                                    wv_sb[t][p0:p0 + kp,
                                             si * 128:(si + 1) * 128],
                                    start=True, stop=True, tile_position=tp)
                                nc.scalar.copy(
                                    stgv[si][:, g * 128:(g + 1) * 128],
                                    cps[:])

                for si, s, off in SCALES:
                    s2 = s * s
                    nc.sync.dma_start(cbuf[off:off + s2, 0:512],
                                      stgk[si][:])
                    nc.sync.dma_start(cbuf[off:off + s2, 512:1536],
                                      stgv[si][:])

                # ---- AllReduce of partial conv outputs (153 KB bf16) ----
                nc.gpsimd.collective_compute(
                    "AllReduce", mybir.AluOpType.add,
                    replica_groups=[list(range(NCORES))],
                    ins=[cbuf.opt()], outs=[arout.opt()])

                # ---- Phase C: per-scale load + bias + relu ----
                art = {}
                for si, s, off in SCALES:
                    s2 = s * s
                    at = rlp.tile([s2, 1536], BF16, tag=f"art{si}")
                    nc.sync.dma_start(at[:], arout[off:off + s2, :])
                    nc.vector.tensor_add(at[:], at[:], bias_sb[si][:])
                    nc.vector.tensor_scalar_max(at[:], at[:], 0.0)
                    art[si] = at

                # ---- Phase D: memory banks via upsample matmuls ----
                # bank_k: [c-half(128), h(2) x m(144)] — lhsT for att
                bkh = [bnk.tile([128, 288], BF16, name=f"bkh{b}", tag=f"bkh{b}")
                       for b in range(B)]
                for b in range(B):
                    for si, s, off in SCALES:
                        s2 = s * s
                        ps_bk = psp.tile([64, 144], F32, tag="ps")
                        for t in range(T):
                            g = t * 2 + b
                            nc.tensor.matmul(
                                ps_bk[:, t * 36:(t + 1) * 36],
                                art[si][:, g * 64:(g + 1) * 64],
                                u_sb[si][:], start=True, stop=True)
                        stg = smm.tile([64, 144], BF16, tag="stgk2")
                        nc.vector.tensor_copy(stg[:], ps_bk[:])
                        nc.sync.dma_start(
                            bkh[b][(si % 2) * 64:(si % 2) * 64 + 64,
                                   (si // 2) * 144:(si // 2) * 144 + 144],
                            stg[:])
                # bank_v m-tiles: bv0 [m 0:128, v 512], bv1 [m 128:144, v 512]
                bv0 = [bnk.tile([128, 512], BF16, name=f"bv0{b}", tag=f"bv0{b}")
                       for b in range(B)]
                bv1 = [bnk.tile([16, 512], BF16, name=f"bv1{b}", tag=f"bv1{b}")
                       for b in range(B)]
                for b in range(B):
                    for t in range(T):
                        g = t * 2 + b
                        ps_bv = psp.tile([36, 512], F32, tag="ps")
                        for si, s, off in SCALES:
                            nc.tensor.matmul(
                                ps_bv[:, si * 128:(si + 1) * 128],
                                u_sb[si][:],
                                art[si][:, 512 + g * 128:512 + (g + 1) * 128],
                                start=True, stop=True)
                        stg = smm.tile([36, 512], BF16, tag="stgv2")
                        nc.vector.tensor_copy(stg[:], ps_bv[:])
                        if t < 3:
                            nc.sync.dma_start(
                                bv0[b][t * 36:(t + 1) * 36, :], stg[:])
                        else:
                            nc.sync.dma_start(bv0[b][108:128, :], stg[0:20, :])
                            nc.sync.dma_start(bv1[b][0:16, :], stg[20:36, :])

                # ---- query loads + query_value passthrough ----
                qkh = [qkp.tile([128, 2 * NS], BF16, name=f"qkh{b}", tag=f"qkh{b}")
                       for b in range(B)]
                for b in range(B):
                    for h in range(2):
                        nc.sync.dma_start(qkh[b][:, h * NS:(h + 1) * NS],
                                          qk[b, 128 * h:128 * (h + 1), :])
                    nc.sync.dma_start(out[b, 0:512, :], qv[b])

                # ---- Phase E: attention (m split 128+16) ----
                for b in range(B):
                    obuf = omp.tile([128, 4 * NS], BF16, name="ob", tag="ob")
                    for ch in range(NCH):
                        cs = slice(ch * NW, (ch + 1) * NW)
                        aps0 = psp.tile([128, NW], F32, tag="ps")
                        aps1 = psp.tile([16, NW], F32, tag="ps")
                        for h in range(2):
                            nc.tensor.matmul(
                                aps0[:],
                                bkh[b][:, h * 144:h * 144 + 128],
                                qkh[b][:, h * NS + ch * NW:h * NS + (ch + 1) * NW],
                                start=(h == 0), stop=(h == 1))
                        for h in range(2):
                            nc.tensor.matmul(
                                aps1[:],
                                bkh[b][:, h * 144 + 128:h * 144 + 144],
                                qkh[b][:, h * NS + ch * NW:h * NS + (ch + 1) * NW],
                                start=(h == 0), stop=(h == 1))
                        ex0 = expp.tile([128, NW], BF16, tag="ex0")
                        ex1 = expp.tile([16, NW], BF16, tag="ex1")
                        nc.scalar.activation(ex0[:], aps0[:], AF.Exp,
                                             scale=1.0 / 16.0)
                        nc.scalar.activation(ex1[:], aps1[:], AF.Exp,
                                             scale=1.0 / 16.0)
                        sums = pss.tile([1, NW], F32, tag="sums")
                        nc.tensor.matmul(sums[:], ones[0:128, 0:1], ex0[:],
                                         start=True, stop=False)
                        nc.tensor.matmul(sums[:], ones[0:16, 0:1], ex1[:],
                                         start=False, stop=True)
                        rrow = smm.tile([1, NW], BF16, tag="rrow")
                        nc.vector.reciprocal(rrow[:], sums[:])
                        bcp = pbc.tile([128, NW], F32, tag="bc")
                        nc.tensor.matmul(bcp[:], ones[0:1, 0:128], rrow[:],
                                         start=True, stop=True)
                        bc_sb = smm.tile([128, NW], BF16, tag="bcs")
                        nc.scalar.copy(bc_sb[:], bcp[:])
                        nc.vector.tensor_mul(ex0[:], ex0[:], bc_sb[:])
                        nc.vector.tensor_mul(ex1[:], ex1[:], bc_sb[0:16, :])
                        for vc in range(4):
                            vs = slice(vc * 128, (vc + 1) * 128)
                            mps = psp.tile([128, NW], F32, tag="ps")
                            nc.tensor.matmul(mps[:], bv0[b][:, vs], ex0[:],
                                             start=True, stop=False)
                            nc.tensor.matmul(mps[:], bv1[b][:, vs], ex1[:],
                                             start=False, stop=True)
                            nc.scalar.copy(
                                obuf[:, vc * NS + ch * NW:vc * NS + (ch + 1) * NW],
                                mps[:])
                    nc.sync.dma_start(
                        out[b, 512:1024, :].rearrange("(v p) n -> p v n", v=4),
                        obuf[:].rearrange("p (v n) -> p v n", v=4))

    nc.compile()
    return nc


def _get_nc(repeats=1, phase='full'):
    key = (repeats, phase)
    if key not in _BUILT:
        _BUILT[key] = _build_nc(repeats, phase)
    return _BUILT[key]


def _host_prep(memory_keys, memory_values, query_key, query_value,
               key_w, key_b, val_w, val_b):
    mk = np.asarray(memory_keys, np.float32).reshape(T, B, CK, HW)
    mv = np.asarray(memory_values, np.float32).reshape(T, B, CV, HW)
    qk = np.asarray(query_key, np.float32).reshape(B, CK, HW)
    qv = np.asarray(query_value, np.float32).reshape(B, CV, HW)
    kw = np.asarray(key_w, np.float32).copy()
    vw = np.asarray(val_w, np.float32).copy()
    kb = np.asarray(key_b, np.float32)
    vb = np.asarray(val_b, np.float32)
    for si, s, off in SCALES:
        kw[:, si] /= BLK[s]
        vw[:, si] /= BLK[s]

    # combined bias constant [50, 512 + 1024]
    bc_host = np.zeros((50, 1536), np.float32)
    for si, s, off in SCALES:
        s2 = s * s
        for t in range(T):
            for b in range(B):
                g = t * 2 + b
                bc_host[off:off + s2, g * 64:(g + 1) * 64] = kb[t, si][None, :]
                bc_host[off:off + s2,
                        512 + g * 128:512 + (g + 1) * 128] = vb[t, si][None, :]
    umats = _upsample_mats()

    mkb = mk.astype(NPBF)
    mvb = mv.astype(NPBF)
    qkb = qk.astype(NPBF)
    qvb = qv.astype(NPBF)
    in_maps = []
    for k in range(NCORES):
        m = {
            "mk": np.ascontiguousarray(mkb[:, :, k * CKC:(k + 1) * CKC, :]),
            "mv": np.ascontiguousarray(mvb[:, :, k * CVC:(k + 1) * CVC, :]),
            "qk": np.ascontiguousarray(qkb[:, :, k * NS:(k + 1) * NS]),
            "qv": np.ascontiguousarray(qvb[:, :, k * NS:(k + 1) * NS]),
            # wk[t, c, si*64+o] = key_w[t, si, o, chunk_c] / blk
            "wk": np.ascontiguousarray(np.tile(
                kw[:, :, :, k * CKC:(k + 1) * CKC]
                .transpose(0, 3, 1, 2).reshape(T, CKC, CK),
                (1, 4, 1))).astype(NPBF),
            "wv": np.ascontiguousarray(np.tile(
                vw[:, :, :, k * CVC:(k + 1) * CVC]
                .transpose(0, 3, 1, 2).reshape(T, CVC, CV),
                (1, 2, 1))).astype(NPBF),
            "bc": bc_host.astype(NPBF),
        }
        for (si, s, off), u in zip(SCALES, umats):
            m[f"u{si}"] = u.astype(NPBF)
        in_maps.append(m)
    return in_maps


def kernel(**inputs):
    nc = _get_nc()
    in_maps = _host_prep(**inputs)
    res = run_bass_kernel_spmd(nc, in_maps, core_ids=list(range(NCORES)),
                               trace=False)
    shards = [np.asarray(res.results[i]["out"]).astype(np.float32)
              for i in range(NCORES)]
    full = np.concatenate(shards, axis=2).reshape(B, 2 * CV, H, W)
    return full
